# revision 18
# baseline (speedup 1.0000x reference)
"""Attention-LSTM pooling kernel for Trainium2 (8 NeuronCores, batch-parallel).

Model (per sample): emb = embedding[x]; h = LSTM(emb); a = tanh(h@W1.T+b1);
alpha = softmax(a@U over seq); ctx = sum_s alpha_s h_s; logit = ctx@W2.T+b2.

Sharding: batch 128 -> 16 samples per core, params replicated.

Per-core on-chip layout (the scan is latency-bound, so everything lives in
transposed "feature-on-partitions" form):
  - tokens ordered t-major: tok = t*16 + b_local
  - hT_all  [128, (S+1, 2, 16)] bf16   (H=256 -> 2 chunks of 128; slot 0 = h_{-1}=0)
  - xpT     [128, (t, 8, 16)] bf16 windows (4H=1024 -> 8 gate chunks of 128)
  - gates psum [128, (8 chunk, 16 b)]; chunk order i,i,f,f,g,g,o,o (host order)

Cell-update algebra (shortens the serial chain): all four gates use ONE tanh
table; sigmoid(x) = (1+tanh(x/2))/2 is folded as
  S = (i'+1)*g' + (f'+1)*c = 2*c_new      (two scalar_tensor_tensor ops + add)
  tanh(c_new) = tanh(0.5*S)               (ACT input scale, free)
  h2 = (o'+1)*tanh(c_new) = 2*h           (scalar_tensor_tensor)
h is stored DOUBLED; W_hh, W1, W2 are pre-scaled x0.5 on the host to
compensate. The o-gate tanh runs in a second ACT off the critical path, and
the scan matmuls are emitted j-outer so the i/f/g ACT starts after 12 of 16.
"""

import os
import sys

sys.path.insert(0, "/opt/trn_rl_repo")

from contextlib import ExitStack

import numpy as np
import ml_dtypes

import concourse.bass as bass
import concourse.bacc as bacc
import concourse.tile as tile
from concourse import mybir
from concourse.bass_utils import run_bass_kernel_spmd

BF16 = mybir.dt.bfloat16
F32 = mybir.dt.float32
I32 = mybir.dt.int32
AF = mybir.ActivationFunctionType
OP = mybir.AluOpType

B, S, V, E, H, A, C = 128, 512, 50000, 256, 256, 128, 2
NCORES = 8
BL = B // NCORES  # 16 samples per core
GC = 8  # gate chunks (4H/128)
KC = 2  # contraction chunks (H/128 == E/128)
EXP_SHIFT = -10.0  # exp(score - 10): guards overflow, softmax-invariant


def build_program(S_=S, V_=V, WIN=64):
    """Emit the bass/tile program for one core (SPMD: all cores identical)."""
    NW = S_ // WIN
    NTOK = S_ * BL // 128  # token tiles of 128
    TPW = NTOK // NW  # token tiles per window
    nc = bacc.Bacc()  # Bacc: TRN2 sync-wait legalization + ACT table loads

    # ---- external I/O (per-core) ----
    table = nc.dram_tensor("table", [V_, E], BF16, kind="ExternalInput").ap()
    idx = nc.dram_tensor("idx", [128, NTOK], I32, kind="ExternalInput").ap()
    wih_t = nc.dram_tensor("wih_t", [E, 4 * H], BF16, kind="ExternalInput").ap()
    whh_t = nc.dram_tensor("whh_t", [H, 4 * H], BF16, kind="ExternalInput").ap()
    bias8 = nc.dram_tensor("bias8", [1, 4 * H], BF16, kind="ExternalInput").ap()
    w1_t = nc.dram_tensor("w1_t", [H, A], BF16, kind="ExternalInput").ap()
    b1T = nc.dram_tensor("b1T", [A, 1], F32, kind="ExternalInput").ap()
    urep = nc.dram_tensor("urep", [A, 128], BF16, kind="ExternalInput").ap()
    w2_t = nc.dram_tensor("w2_t", [H, C], F32, kind="ExternalInput").ap()
    b2row = nc.dram_tensor("b2row", [1, C], F32, kind="ExternalInput").ap()
    ident = nc.dram_tensor("ident", [128, 128], BF16, kind="ExternalInput").ap()
    logits = nc.dram_tensor("logits", [BL, C], F32, kind="ExternalOutput").ap()

    with tile.TileContext(nc) as tc, ExitStack() as ctx:
        const = ctx.enter_context(tc.tile_pool(name="const", bufs=1))
        state = ctx.enter_context(tc.tile_pool(name="state", bufs=1))
        win = ctx.enter_context(tc.tile_pool(name="win", bufs=2))
        # one slot per token tile: indirect DMAs only support a single sync
        # wait, so gather slots must never be reused (reuse adds a PE wait)
        embp = ctx.enter_context(tc.tile_pool(name="embp", bufs=NTOK))
        work = ctx.enter_context(tc.tile_pool(name="work", bufs=3))
        pg = ctx.enter_context(tc.tile_pool(name="pg", bufs=3, space="PSUM"))
        pb = ctx.enter_context(tc.tile_pool(name="pb", bufs=2, space="PSUM"))
        ptr_pool = ctx.enter_context(tc.tile_pool(name="ptr", bufs=2, space="PSUM"))
        plog_pool = ctx.enter_context(tc.tile_pool(name="plog", bufs=1, space="PSUM"))

        # ---- load constants ----
        def load_const(shape, dt, src, tag):
            t = const.tile(shape, dt, tag=tag)
            nc.sync.dma_start(t[:], src)
            return t

        whh_sb = [
            load_const([128, 4 * H], BF16, whh_t[k * 128 : (k + 1) * 128, :], f"whh{k}")
            for k in range(KC)
        ]
        wih_sb = [
            load_const([128, 4 * H], BF16, wih_t[k * 128 : (k + 1) * 128, :], f"wih{k}")
            for k in range(KC)
        ]
        bias_sb = load_const([1, 4 * H], BF16, bias8[:], "bias")
        w1_sb = [
            load_const([128, A], BF16, w1_t[k * 128 : (k + 1) * 128, :], f"w1{k}")
            for k in range(KC)
        ]
        b1_sb = load_const([A, 1], F32, b1T[:], "b1")
        urep_sb = load_const([A, 128], BF16, urep[:], "urep")
        w2_sb = [
            load_const([128, C], F32, w2_t[k * 128 : (k + 1) * 128, :], f"w2{k}")
            for k in range(KC)
        ]
        b2_sb = load_const([1, C], F32, b2row[:], "b2")
        id_sb = load_const([128, 128], BF16, ident[:], "ident")
        idx_sb = load_const([128, NTOK], I32, idx[:], "idx")
        ones_sb = const.tile([1, BL], F32, tag="ones")
        nc.vector.memset(ones_sb[:], 1.0)
        eshift_sb = const.tile([128, 1], F32, tag="eshift")
        nc.vector.memset(eshift_sb[:], EXP_SHIFT)
        ones512 = const.tile([1, 512], BF16, tag="ones512")
        nc.vector.memset(ones512[:], 1.0)

        # ---- persistent state ----
        hT_all = state.tile([128, (S_ + 1) * 2 * BL], BF16, tag="hT")
        hT = hT_all[:].rearrange("p (t c b) -> p t c b", t=S_ + 1, c=2, b=BL)
        nc.vector.memset(hT[:, 0], 0.0)  # h_{-1} = 0
        c_st = state.tile([128, 2 * BL], F32, tag="c")  # [(c chunk, b)]
        c_v = c_st[:].rearrange("p (c b) -> p c b", c=2)
        nc.vector.memset(c_st[:], 0.0)
        # attention accumulators (pooling runs per-window, inside the scan)
        ctx_acc = state.tile([128, 2 * BL], F32, tag="ctxa")
        nc.vector.memset(ctx_acc[:], 0.0)
        esum_acc = state.tile([128, BL], F32, tag="esum")
        nc.vector.memset(esum_acc[:], 0.0)

        NCH = WIN * BL // 512  # 512-token chunks per window
        TCH = 512 // BL  # timesteps per chunk

        # ---- window producer plans: gather -> transpose -> xp GEMM.
        # Emitted interleaved with the PREVIOUS window's scan steps so the
        # producer matmuls fill the scan's idle PE gaps instead of bursting
        # serially at each window boundary.
        TPC = TPW // NCH  # token tiles per 512-col chunk

        def make_plan(w):
            embT = win.tile([128, KC * WIN * BL], BF16, tag="embT")
            embT_v = embT[:].rearrange("p (k n) -> p k n", k=KC)
            xpT = win.tile([128, WIN * GC * BL], BF16, tag="xpT")
            xpT_v = xpT[:].rearrange("p (t g b) -> p t g b", t=WIN, g=GC)
            # interleave so each 512-col chunk n becomes ready as early as
            # possible: its gathers, then its 8 gate GEMMs
            ops = []
            if w == 0:
                # startup: 128-col sub-GEMMs so the scan can begin after one
                # gather + 8 small GEMMs (each tok tile covers 8 timesteps)
                for s in range(TPC):
                    ops += [("tok", s)]
                    ops += [("gemm8", j, s) for j in range(GC)]
                ops += [("tok", TPC + j) for j in range(TPC)]
                ops += [("gemm", j, 1) for j in range(GC)]
            else:
                for n in range(NCH):
                    ops += [("tok", n * TPC + j) for j in range(TPC)]
                    ops += [("gemm", j, n) for j in range(GC)]
            return {"w": w, "embT_v": embT_v, "xpT_v": xpT_v, "ops": ops}

        def emit_producer_op(plan):
            if not plan["ops"]:
                return
            op = plan["ops"].pop(0)
            w, embT_v, xpT_v = plan["w"], plan["embT_v"], plan["xpT_v"]
            if op[0] == "tok":
                j = op[1]
                emb_sb = embp.tile([128, E], BF16, tag="emb")
                nc.gpsimd.indirect_dma_start(
                    out=emb_sb[:],
                    out_offset=None,
                    in_=table[:],
                    in_offset=bass.IndirectOffsetOnAxis(
                        ap=idx_sb[:, w * TPW + j : w * TPW + j + 1], axis=0
                    ),
                )
                for k in range(KC):
                    ptr = ptr_pool.tile([128, 128], BF16, tag="tr")
                    nc.tensor.transpose(
                        ptr[:], emb_sb[:, k * 128 : (k + 1) * 128], id_sb[:]
                    )
                    nc.any.tensor_copy(
                        embT_v[:, k, j * 128 : (j + 1) * 128], ptr[:]
                    )
            elif op[0] == "gemm8":
                # startup-only 128-col (8-timestep) sub-GEMM of chunk n=0
                _, j, s = op
                pxp = pg.tile([128, GC * BL], F32, tag="g")
                for k in range(KC):
                    nc.tensor.matmul(
                        pxp[:],
                        wih_sb[k][:, j * 128 : (j + 1) * 128],
                        embT_v[:, k, s * 128 : (s + 1) * 128],
                        start=(k == 0),
                        stop=False,
                        skip_group_check=True,
                    )
                nc.tensor.matmul(
                    pxp[:], bias_sb[:, j * 128 : (j + 1) * 128],
                    ones512[:, 0:128], start=False, stop=True,
                    skip_group_check=True,
                )
                nc.any.tensor_copy(
                    xpT_v[:, s * 8 : (s + 1) * 8, j, :],
                    pxp[:].rearrange("p (t b) -> p t b", b=BL),
                )
            else:
                _, j, n = op
                pxp = pb.tile([128, 512], F32, tag="big")
                for k in range(KC):
                    nc.tensor.matmul(
                        pxp[:],
                        wih_sb[k][:, j * 128 : (j + 1) * 128],
                        embT_v[:, k, n * 512 : (n + 1) * 512],
                        start=(k == 0),
                        stop=False,
                    )
                # bias via K=1 matmul (keeps evac a pure Copy: no ACT
                # Identity-with-bias, which would thrash the table set)
                nc.tensor.matmul(
                    pxp[:], bias_sb[:, j * 128 : (j + 1) * 128], ones512[:],
                    start=False, stop=True,
                )
                nc.any.tensor_copy(
                    xpT_v[:, n * TCH : (n + 1) * TCH, j, :],
                    pxp[:].rearrange("p (t b) -> p t b", b=BL),
                )

        # ---- attention plans: per-window pooling partials, emitted one op
        # per scan step of the FOLLOWING window so they fill engine idle time
        # instead of stalling the in-order queues at each window boundary.
        def make_att_plan(w):
            t0 = w * WIN
            aw = win.tile([128, WIN * BL], BF16, tag="aw")
            ew = win.tile([128, WIN * BL], F32, tag="ew")
            ops = []
            for n in range(NCH):
                ops += [("aw_mm", n), ("aw_act", n), ("ew_mm", n),
                        ("ew_act", n), ("esum", n)]
                for cch in range(2):
                    ops += [("ctx_mul", n, cch), ("ctx_red", n, cch)]
            return {"t0": t0, "aw": aw, "ew": ew, "ops": ops,
                    "pa": None, "pe": None, "htld": {}}

        def emit_att_op(plan):
            if not plan["ops"]:
                return
            op = plan["ops"].pop(0)
            t0, aw, ew = plan["t0"], plan["aw"], plan["ew"]
            kind, n = op[0], op[1]
            c0, c1 = n * 512, (n + 1) * 512
            ts0, ts1 = t0 + 1 + n * TCH, t0 + 1 + (n + 1) * TCH
            if kind == "aw_mm":
                pa = pb.tile([128, 512], F32, tag="big")
                for k in range(KC):
                    nc.tensor.matmul(
                        pa[:], w1_sb[k][:], hT[:, ts0:ts1, k, :],
                        start=(k == 0), stop=(k == KC - 1),
                    )
                plan["pa"] = pa
            elif kind == "aw_act":
                nc.scalar.activation(
                    aw[:, c0:c1], plan["pa"][:], AF.Tanh, bias=b1_sb[:],
                )
            elif kind == "ew_mm":
                pe_ = pb.tile([128, 512], F32, tag="big")
                nc.tensor.matmul(
                    pe_[:], urep_sb[:], aw[:, c0:c1], start=True, stop=True,
                )
                plan["pe"] = pe_
            elif kind == "ew_act":
                nc.scalar.activation(
                    ew[:, c0:c1], plan["pe"][:], AF.Exp, bias=eshift_sb[:],
                )
            elif kind == "esum":
                psum_w = work.tile([128, BL], F32, tag="psw")
                nc.vector.tensor_reduce(
                    out=psum_w[:],
                    in_=ew[:, c0:c1].rearrange("p (t b) -> p b t", t=TCH),
                    axis=mybir.AxisListType.X, op=OP.add,
                )
                nc.gpsimd.tensor_tensor(
                    out=esum_acc[:], in0=esum_acc[:], in1=psum_w[:], op=OP.add
                )
            elif kind == "ctx_mul":
                cch = op[2]
                htld = work.tile([128, TCH * BL], F32, tag=f"htld{cch}")
                nc.gpsimd.tensor_tensor(
                    out=htld[:].rearrange("p (t b) -> p t b", t=TCH),
                    in0=hT[:, ts0:ts1, cch, :],
                    in1=ew[:, c0:c1].rearrange("p (t b) -> p t b", t=TCH),
                    op=OP.mult,
                )
                plan["htld"][cch] = htld
            else:  # ctx_red
                cch = op[2]
                pctx_w = work.tile([128, BL], F32, tag="pcw")
                nc.vector.tensor_reduce(
                    out=pctx_w[:],
                    in_=plan["htld"][cch][:].rearrange(
                        "p (t b) -> p b t", t=TCH),
                    axis=mybir.AxisListType.X, op=OP.add,
                )
                nc.gpsimd.tensor_tensor(
                    out=ctx_acc[:, cch * BL : (cch + 1) * BL],
                    in0=ctx_acc[:, cch * BL : (cch + 1) * BL],
                    in1=pctx_w[:], op=OP.add,
                )

        # window 0: emit only the first sub-chunk's producers up front
        # (1 gather + 8 sub-GEMMs); the rest pace into the scan
        cur = make_plan(0)
        for _ in range(1 + GC):
            emit_producer_op(cur)

        HB = BL
        att = None  # attention plan of the previous window
        for w in range(NW):
            t0 = w * WIN
            xpT_v = cur["xpT_v"]
            nxt = make_plan(w + 1) if w + 1 < NW else None

            # ---- LSTM scan over this window ----
            for tl in range(WIN):
                t = t0 + tl
                # pace producers + attention partials into the scan's engine
                # idle gaps (current-window producer leftovers are the most
                # urgent, 2/step; then alternate attention / next window)
                if cur["ops"]:
                    emit_producer_op(cur)
                    if cur["ops"]:
                        emit_producer_op(cur)
                elif att is not None and att["ops"] and tl % 2 == 1:
                    emit_att_op(att)
                elif nxt is not None and nxt["ops"]:
                    emit_producer_op(nxt)
                elif att is not None and att["ops"]:
                    emit_att_op(att)
                pgate = pg.tile([128, GC * HB], F32, tag="g")
                # seed psum with xp_t via identity matmul (keeps the
                # xp-add off the serial chain), then accumulate W_hh @ h
                nc.tensor.matmul(
                    pgate[:],
                    id_sb[:],
                    xpT_v[:, tl, :, :],
                    start=True,
                    stop=False,
                    skip_group_check=True,
                )
                # j-outer: i/f/g chunks (j=0..5) complete first so their
                # ACT starts while the o-chunk matmuls still run
                for j in range(GC):
                    for k in range(KC):
                        nc.tensor.matmul(
                            pgate[:, j * HB : (j + 1) * HB],
                            whh_sb[k][:, j * 128 : (j + 1) * 128],
                            hT[:, t, k, :],
                            start=False,
                            stop=(k == KC - 1),
                            skip_group_check=True,
                        )
                gact = work.tile([128, GC * HB], F32, tag="gact")
                # chunk cols (xHB): i=0:2, f=2:4, g=4:6, o=6:8; i,f,o rows
                # pre-scaled x0.5 so one tanh table serves all gates
                nc.scalar.activation(gact[:, 0 : 6 * HB], pgate[:, 0 : 6 * HB], AF.Tanh)
                nc.scalar.activation(gact[:, 6 * HB : 8 * HB], pgate[:, 6 * HB : 8 * HB], AF.Tanh)
                ig2 = work.tile([128, 2 * HB], F32, tag="ig2")
                nc.vector.scalar_tensor_tensor(
                    out=ig2[:], in0=gact[:, 0 : 2 * HB], scalar=1.0,
                    in1=gact[:, 4 * HB : 6 * HB], op0=OP.add, op1=OP.mult,
                )
                fc2 = work.tile([128, 2 * HB], F32, tag="fc2")
                nc.vector.scalar_tensor_tensor(
                    out=fc2[:].rearrange("p (c b) -> p c b", c=2),
                    in0=gact[:, 2 * HB : 4 * HB].rearrange("p (c b) -> p c b", c=2),
                    scalar=1.0,
                    in1=c_v[:, :, :],
                    op0=OP.add, op1=OP.mult,
                )
                s2 = work.tile([128, 2 * HB], F32, tag="s2")  # S = 2*c_new
                nc.vector.tensor_tensor(
                    out=s2[:], in0=ig2[:], in1=fc2[:], op=OP.add,
                )
                # plain c for next step's fc2 (off the tanh path)
                nc.vector.tensor_scalar_mul(c_st[:], s2[:], 0.5)
                tc_sb = work.tile([128, 2 * HB], F32, tag="tc")
                nc.scalar.activation(tc_sb[:], s2[:], AF.Tanh, scale=0.5)
                # h2 = (o'+1)*tanh(c) = 2h; W_hh/W1/W2 pre-scaled x0.5
                nc.vector.scalar_tensor_tensor(
                    out=hT[:, t + 1, :, :],
                    in0=gact[:, 6 * HB : 8 * HB].rearrange("p (c b) -> p c b", c=2),
                    scalar=1.0,
                    in1=tc_sb[:].rearrange("p (c b) -> p c b", c=2),
                    op0=OP.add, op1=OP.mult,
                )

            # flush any unpaced ops, then hand off plans
            if att is not None:
                while att["ops"]:
                    emit_att_op(att)
            if nxt is not None:
                while nxt["ops"]:
                    emit_producer_op(nxt)
            att = make_att_plan(w)
            cur = nxt

        # tail: attention partials of the final window
        while att["ops"]:
            emit_att_op(att)

        # ---- finish: ctx = ctx_acc / esum ; logits = ctx @ W2.T + b2 ----
        rsum = work.tile([128, BL], F32, tag="rsum")
        nc.vector.reciprocal(rsum[:], esum_acc[:])
        ctxn = work.tile([128, 2 * BL], F32, tag="ctxn")
        for cch in range(2):
            nc.vector.tensor_tensor(
                out=ctxn[:, cch * BL : (cch + 1) * BL],
                in0=ctx_acc[:, cch * BL : (cch + 1) * BL],
                in1=rsum[:],
                op=OP.mult,
            )
        plog = plog_pool.tile([BL, C], F32, tag="plog")
        for cch in range(2):
            nc.tensor.matmul(
                plog[:], ctxn[:, cch * BL : (cch + 1) * BL], w2_sb[cch][:],
                start=(cch == 0), stop=False,
            )
        nc.tensor.matmul(plog[:], ones_sb[:], b2_sb[:], start=False, stop=True)
        out_sb = work.tile([BL, C], F32, tag="outsb")
        nc.vector.tensor_copy(out_sb[:], plog[:])
        nc.sync.dma_start(logits[:], out_sb[:])

    nc.finalize()
    return nc


def prep_inputs(x, embedding, W_ih, W_hh, b_ih, b_hh, W1, b1, U, W2, b2, S_=S, V_=V):
    """Host-side parameter prep + per-core input maps."""
    bf = ml_dtypes.bfloat16
    # gates stay in torch order [i,f,g,o]; i,f,o rows pre-scaled x0.5:
    # sigmoid(x) = (1 + tanh(x/2))/2, so the whole kernel needs only the
    # {tanh, exp} ACT table set (no mid-scan reloads)
    gsc = np.ones((4 * H, 1), np.float32)
    gsc[: 2 * H] = 0.5
    gsc[3 * H :] = 0.5
    wih_r = np.asarray(W_ih) * gsc  # [4H, E]
    # extra x0.5 on W_hh: the stored h is doubled (h2 = (o'+1)*tanh(c))
    whh_r = np.asarray(W_hh) * gsc * 0.5
    bias_r = (np.asarray(b_ih) + np.asarray(b_hh)) * gsc[:, 0]  # [4H]

    common = {
        "table": np.ascontiguousarray(np.asarray(embedding).astype(bf)),
        "wih_t": np.ascontiguousarray(wih_r.T.astype(bf)),  # [E, 4H]
        "whh_t": np.ascontiguousarray(whh_r.T.astype(bf)),  # [H, 4H]
        "bias8": np.ascontiguousarray(bias_r.reshape(1, 4 * H).astype(bf)),
        "w1_t": np.ascontiguousarray((np.asarray(W1) * 0.5).T.astype(bf)),  # [H, A]
        "b1T": np.ascontiguousarray(np.asarray(b1).reshape(A, 1).astype(np.float32)),
        "urep": np.ascontiguousarray(
            np.repeat(np.asarray(U).astype(np.float32), 128, axis=1).astype(bf)
        ),
        "w2_t": np.ascontiguousarray((np.asarray(W2) * 0.5).T.astype(np.float32)),  # [H, C]
        "b2row": np.ascontiguousarray(np.asarray(b2).reshape(1, C).astype(np.float32)),
        "ident": np.eye(128, dtype=np.float32).astype(bf),
    }
    x = np.asarray(x)
    in_maps = []
    for c in range(NCORES):
        xs = x[c * BL : (c + 1) * BL]  # [BL, S]
        # token order t-major: tok = t*BL + b ; tile j rows p -> tok = j*128+p
        toks = xs.T.reshape(-1)  # [S*BL]
        ntok = S_ * BL // 128
        idx_np = toks.reshape(ntok, 128).T.copy().astype(np.int32)  # [128, NTOK]
        in_maps.append({**common, "idx": idx_np})
    return in_maps


_prog_cache = {}


def kernel(x, embedding, W_ih, W_hh, b_ih, b_hh, W1, b1, U, W2, b2):
    key = "full"
    if key not in _prog_cache:
        _prog_cache[key] = build_program()
    nc = _prog_cache[key]
    in_maps = prep_inputs(x, embedding, W_ih, W_hh, b_ih, b_hh, W1, b1, U, W2, b2)
    res = run_bass_kernel_spmd(nc, in_maps, list(range(NCORES)))
    kernel.last_results = res  # exec_time_ns/profile when BASS_TRACE=1
    out = np.concatenate([res.results[c]["logits"] for c in range(NCORES)], axis=0)
    return out.astype(np.float32)


if __name__ == "__main__":
    import reference

    inputs = {k: np.asarray(v) for k, v in reference.setup_inputs().items()}
    got = kernel(**inputs)
    exp = np.asarray(reference.reference(**inputs))
    rel = np.abs(got - exp).max() / np.abs(exp).max()
    print("Relative error:", rel)

